# revision 41
# baseline (speedup 1.0000x reference)
"""Self-contained Trainium2 kernel for nn_Attention_24799141167815.

Cosine-similarity attention (Swin-v2 style) with continuous position bias.
Data-parallel over batch B=8 across 8 NeuronCores (core b handles batch b).

Design (vs original baseline):
  - rel-bias is RAW (not exp'd), log2e-scaled, fp8e4, and added into the
    scores PSUM via an identity-stationary matmul at the start of each
    accumulation group; the ACT Exp(scale=ln2) converts back. This removes
    the post-exp multiply chain entirely and halves the bias DMA traffic.
  - score matmuls for the two heads of a feature block are row-packed
    (head A on partitions 0-63, head B on 64-127, issued back-to-back so
    both 64-deep matmuls run concurrently in the PE array).
  - qb-merged attention groups: the scores psum is [128, 1024] (both
    512-wide q blocks of one key tile) so each ACT Exp covers FD=1024,
    halving the per-op ACT overhead on the critical engine.
  - qsT/knT/x/weights in fp16 (not bf16) for 4x better score precision.
  - reciprocals via reciprocal_approx_fast (~1 cyc/elem, not 8/elem),
    always from SBUF (custom DVE ops misread PSUM at partition offsets).
  - emission order keeps every engine queue dense: attention for head
    pair 0 is emitted right after its dependencies, with later head
    pairs' QKV/norms interleaved between groups.
"""

import os
import numpy as np
import ml_dtypes

import concourse.bass as bass
import concourse.mybir as mybir
import concourse.tile as tile
from concourse import bacc
from concourse.bass_utils import run_bass_kernel_spmd

F32 = mybir.dt.float32
BF16 = mybir.dt.bfloat16
FP16 = mybir.dt.float16
FP8 = mybir.dt.float8e4
AF = mybir.ActivationFunctionType
ALU = mybir.AluOpType

B, N, C = 8, 1024, 512
H, HD = 8, 64
NT = N // 128     # 8 key tiles
CB = C // 128     # 4 cin blocks
QB = 2            # q blocks of 512
NB_BF16 = np.dtype(ml_dtypes.bfloat16)
NB_FP16 = np.dtype(np.float16)
NB_FP8 = np.dtype(ml_dtypes.float8_e4m3)
LOG2E = float(np.log2(np.e))
LN2 = float(np.log(2.0))

_CACHE = {}


def _build(reps=1):
    nc = bacc.Bacc("TRN2", target_bir_lowering=False)

    xT_d = nc.declare_dram_parameter("xT", [C, N], FP16, isOutput=False)
    wqkT_d = nc.declare_dram_parameter("wqkT", [C, 2 * C], FP16, isOutput=False)
    wvT_d = nc.declare_dram_parameter("wvT", [C, C], FP16, isOutput=False)
    qkb_d = nc.declare_dram_parameter("qkb", [2 * C, 1], F32, isOutput=False)
    vbrow_d = nc.declare_dram_parameter("vbrow", [1, C], F32, isOutput=False)
    qesbd_d = nc.declare_dram_parameter("qesbd", [C, 2], FP16, isOutput=False)
    sclq_d = nc.declare_dram_parameter("sclq", [2, CB], F32, isOutput=False)
    projwT_d = nc.declare_dram_parameter("projwT", [C, C], FP16, isOutput=False)
    projbrow_d = nc.declare_dram_parameter("projbrow", [1, C], F32, isOutput=False)
    biasT_d = nc.declare_dram_parameter("biasT", [H, N, N], FP8, isOutput=False)
    selb_d = nc.declare_dram_parameter("selb", [2, 128], FP16, isOutput=False)
    bsum_d = nc.declare_dram_parameter("bsum", [128, 2], FP16, isOutput=False)
    identf8_d = nc.declare_dram_parameter("identf8", [128, 128], FP8, isOutput=False)
    ones64_d = nc.declare_dram_parameter("ones64", [1, 64], BF16, isOutput=False)
    out_d = nc.declare_dram_parameter("out", [N, C], F32, isOutput=True)

    with tile.TileContext(nc) as tc:
        with (
            tc.tile_pool(name="persist", bufs=1) as persist,
            tc.tile_pool(name="sqp", bufs=2) as sqp,
            tc.tile_pool(name="ebias", bufs=3) as ebias,
            tc.tile_pool(name="expt", bufs=2) as expt_pool,
            tc.tile_pool(name="small", bufs=2) as small,
            tc.tile_pool(name="osbp", bufs=2) as osbp,
            tc.tile_pool(name="ps_big", bufs=3, space="PSUM") as ps_big,
            tc.tile_pool(name="ps_av", bufs=2, space="PSUM") as ps_av,
        ):
            # ---------------- load constants / weights ----------------
            # xT per cb chunk so the first matmul (which contracts cb=0
            # first) can start as soon as possible
            xT = persist.tile([128, CB, N], FP16, tag="xT")
            nc.sync.dma_start(
                out=xT[:, 0:2, :],
                in_=xT_d.rearrange("(cb p) n -> p cb n", p=128)[:, 0:2, :])
            nc.scalar.dma_start(
                out=xT[:, 2:4, :],
                in_=xT_d.rearrange("(cb p) n -> p cb n", p=128)[:, 2:4, :])
            wqkT = persist.tile([128, CB, 2 * C], FP16, tag="wqkT")
            wvT = persist.tile([128, CB, C], FP16, tag="wvT")
            for i, fb in enumerate([0, CB, 1, CB + 1, 2, CB + 2, 3, CB + 3]):
                nc.sync.dma_start(
                    out=wqkT[:, :, fb * 128:(fb + 1) * 128],
                    in_=wqkT_d.rearrange("(cb p) f -> p cb f", p=128)[
                        :, :, fb * 128:(fb + 1) * 128])
                if i == 1:
                    nc.sync.dma_start(
                        out=wvT,
                        in_=wvT_d.rearrange("(cb p) f -> p cb f", p=128))
            qkb = persist.tile([128, 2 * CB], F32, tag="qkb")
            nc.sync.dma_start(
                out=qkb, in_=qkb_d.rearrange("(fb p) one -> p (fb one)", p=128))
            sclq = persist.tile([2, CB], F32, tag="sclq")
            nc.sync.dma_start(out=sclq, in_=sclq_d[:])
            selb = persist.tile([2, 128], FP16, tag="selb")
            nc.sync.dma_start(out=selb, in_=selb_d[:])
            bsum = persist.tile([128, 2], FP16, tag="bsum")
            nc.sync.dma_start(out=bsum, in_=bsum_d[:])
            identf8 = persist.tile([128, 128], FP8, tag="identf8")
            nc.sync.dma_start(out=identf8, in_=identf8_d[:])
            ones64 = persist.tile([1, 64], BF16, tag="ones64")
            nc.sync.dma_start(out=ones64, in_=ones64_d[:])
            vb_bc = persist.tile([128, C], F32, tag="vb_bc")
            nc.sync.dma_start(out=vb_bc, in_=vbrow_d[:].to_broadcast((128, C)))
            qesbd = persist.tile([128, CB, 2], FP16, tag="qesbd")
            nc.sync.dma_start(
                out=qesbd, in_=qesbd_d.rearrange("(cb p) s -> p cb s", p=128))

            for rep in range(reps):
                qkT = persist.tile([128, 2 * CB, N], FP16, tag="qkT")
                qsT = persist.tile([128, CB, N], FP16, tag="qsT")
                knT = persist.tile([128, CB, N], FP16, tag="knT")
                v_sb = persist.tile([128, NT, H, HD + 1], BF16, tag="v_sb")
                qekn = persist.tile([128, CB, NT, 2], F32, tag="qekn")
                outhT = persist.tile([128, CB, N], FP16, tag="outhT")
                nrms = [None] * (2 * CB)

                def qkv(hp):
                    # q (fb=hp) and k (fb=CB+hp) projections, DVE eviction
                    for half in range(2):
                        fb = half * CB + hp
                        for qb in range(QB):
                            ps = ps_big.tile(
                                [128, 1024], F32, tag="ps_big",
                                name=f"psqkv{hp}{half}{qb}")
                            for cb in range(CB):
                                nc.tensor.matmul(
                                    ps[:, 0:512],
                                    wqkT[:, cb, fb * 128:(fb + 1) * 128],
                                    xT[:, cb, qb * 512:(qb + 1) * 512],
                                    start=(cb == 0), stop=(cb == CB - 1),
                                )
                            nc.vector.tensor_scalar(
                                out=qkT[:, fb, qb * 512:(qb + 1) * 512],
                                in0=ps[:, 0:512], scalar1=qkb[:, fb:fb + 1],
                                scalar2=None, op0=ALU.add)

                def vproj():
                    nc.vector.memset(v_sb[:, :, :, HD:HD + 1], 1.0)
                    for tb in range(NT):
                        ps = ps_big.tile([128, 1024], F32, tag="ps_big",
                                         name=f"psv{tb}")
                        for cb in range(CB):
                            nc.tensor.matmul(
                                ps[:, 0:512],
                                xT[:, cb, tb * 128:(tb + 1) * 128],
                                wvT[:, cb, :],
                                start=(cb == 0), stop=(cb == CB - 1),
                            )
                        nc.vector.tensor_add(
                            v_sb[:, tb, :, 0:HD],
                            ps[:, 0:512].rearrange("p (h d) -> p h d", h=H),
                            vb_bc.rearrange("p (h d) -> p h d", h=H),
                        )

                def norms_tensor(hp):
                    # squared sums -> sqrt -> 1/x -> clamp+scale, per q/k half
                    for half in range(2):
                        fb = half * CB + hp
                        sq = sqp.tile([128, N], FP16, tag="sq",
                                      name=f"sq{hp}{half}")
                        nc.vector.tensor_mul(sq, qkT[:, fb, :], qkT[:, fb, :])
                        nrm = sqp.tile([2, N], F32, tag="nrm", bufs=1,
                                       name=f"nrm{hp}{half}")
                        for qb in range(QB):
                            pss = ps_big.tile([128, 1024], F32, tag="ps_big",
                                              name=f"psn{hp}{half}{qb}")
                            nc.tensor.matmul(
                                pss[0:2, 0:512], bsum,
                                sq[:, qb * 512:(qb + 1) * 512],
                                start=True, stop=True)
                            nc.scalar.activation(
                                out=nrm[:, qb * 512:(qb + 1) * 512],
                                in_=pss[0:2, 0:512],
                                func=AF.Sqrt, bias=0.0, scale=1.0)
                        rr = sqp.tile([2, N], F32, tag="rr", bufs=1,
                                      name=f"rr{hp}{half}")
                        nc.vector.reciprocal_approx_fast(rr, nrm)
                        rr16 = sqp.tile([2, N], FP16, tag="rr16",
                                        name=f"rr16{hp}{half}")
                        nc.vector.tensor_scalar(
                            out=rr16, in0=rr,
                            scalar1=1e12, scalar2=(
                                sclq[:, hp:hp + 1] if half == 0 else 1.0),
                            op0=ALU.min, op1=ALU.mult)
                        nrms[2 * hp + half] = rr16

                def scaleqk(hp):
                    # broadcast norms to 128 partitions; scale features
                    for half, dst in ((0, qsT), (1, knT)):
                        fb = half * CB + hp
                        rr16 = nrms[2 * hp + half]
                        for qb in range(QB):
                            psb = ps_big.tile([128, 1024], F32, tag="ps_big",
                                              name=f"psb{hp}{half}{qb}")
                            nc.tensor.matmul(
                                psb[:, 0:512], selb,
                                rr16[:, qb * 512:(qb + 1) * 512],
                                start=True, stop=True)
                            nc.vector.tensor_mul(
                                dst[:, hp, qb * 512:(qb + 1) * 512],
                                qkT[:, fb, qb * 512:(qb + 1) * 512],
                                psb[:, 0:512])

                def qekn_calc(hp):
                    psq = ps_big.tile([128, 1024], F32, tag="ps_big",
                                      name=f"psq{hp}")
                    for kt in range(NT):
                        nc.tensor.matmul(
                            psq[:, kt * 2:kt * 2 + 2],
                            knT[:, hp, kt * 128:(kt + 1) * 128],
                            qesbd[:, hp, :],
                            start=True, stop=True)
                    nc.vector.tensor_copy(
                        qekn[:, hp, :, :].rearrange("p a s -> p (a s)"),
                        psq[:, 0:2 * NT])

                bias_tiles = {}

                def prefetch_bias(h):
                    # bias DMAs ride the scalar (ACT) hwdge queue so the big
                    # fp8 streams never block weight loads on the sync queue
                    bt = ebias.tile([128, NT, 1024], FP8, tag="bias",
                                    name=f"bias{h}")
                    nc.scalar.dma_start(
                        out=bt,
                        in_=biasT_d[h].rearrange("(kt p) q -> p kt q", p=128))
                    bias_tiles[h] = bt

                pav_tiles = {}
                eT_tiles = {}

                def group_main(h):
                    # attention for one head, both q blocks at once
                    hp, sub = h // 2, h % 2
                    po = sub * 64
                    bias_t = bias_tiles.pop(h)
                    eT = expt_pool.tile([128, NT, 1024], BF16, tag="eT",
                                        name=f"eT{h}")
                    eT_tiles[h] = eT
                    pavs = [ps_av.tile([HD + 1, 512], F32, tag="ps_av",
                                       name=f"pav{h}{i}") for i in range(2)]
                    pav_tiles[h] = pavs

                    def scores(kt):
                        ps = ps_big.tile([128, 1024], F32, tag="ps_big",
                                         name=f"pssc{h}{kt}")
                        for qb in range(QB):
                            nc.tensor.matmul(
                                ps[:, qb * 512:(qb + 1) * 512], identf8,
                                bias_t[:, kt, qb * 512:(qb + 1) * 512],
                                start=True, stop=False)
                        for qb in range(QB):
                            nc.tensor.matmul(
                                ps[:, qb * 512:(qb + 1) * 512],
                                knT[:, hp, kt * 128:(kt + 1) * 128][po:po + 64],
                                qsT[:, hp, qb * 512:(qb + 1) * 512][po:po + 64],
                                start=False, stop=True,
                            )
                        nc.scalar.activation(
                            out=eT[:, kt, :], in_=ps, func=AF.Exp,
                            bias=qekn[:, hp, kt, sub:sub + 1], scale=LN2)

                    def av(kt):
                        for qb in range(QB):
                            nc.tensor.matmul(
                                pavs[qb],
                                v_sb[:, kt, h, :],
                                eT[:, kt, qb * 512:(qb + 1) * 512],
                                start=(kt == 0), stop=(kt == NT - 1),
                            )

                    # AV lags scores by 3 kt so the previous group's division
                    # (which holds the pav slots) has time to finish
                    for kt in range(NT):
                        scores(kt)
                        if kt >= 3:
                            av(kt - 3)
                    for kt in range(NT - 3, NT):
                        av(kt)

                def group_div(h, qb):
                    # division for one (head, q-block): out = num * recip(den)
                    hp, sub = h // 2, h % 2
                    po = sub * 64
                    pav = pav_tiles[h][qb]
                    den = small.tile([1, 512], F32, tag="dens",
                                     name=f"den{h}{qb}")
                    nc.vector.tensor_copy(den, pav[HD:HD + 1, :])
                    rrec = small.tile([1, 512], F32, tag="rrec",
                                      name=f"rrec{h}{qb}")
                    nc.vector.reciprocal_approx_fast(rrec, den)
                    rrb = small.tile([HD, 512], F32, tag="rrb",
                                     name=f"rrb{h}{qb}")
                    nc.gpsimd.partition_broadcast(rrb, rrec)
                    nc.vector.scalar_tensor_tensor(
                        out=outhT[:, hp, qb * 512:(qb + 1) * 512][po:po + 64],
                        in0=pav[0:HD, :], scalar=1.0, in1=rrb,
                        op0=ALU.mult, op1=ALU.mult)

                def proj(tb):
                    ps = ps_big.tile([128, 1024], F32, tag="ps_big",
                                     name=f"pso{tb}")
                    for fb in range(CB):
                        nc.tensor.matmul(
                            ps[:, 0:512],
                            outhT[:, fb, tb * 128:(tb + 1) * 128],
                            projwT[:, fb, :],
                            start=(fb == 0), stop=(fb == CB - 1),
                        )
                    osb = osbp.tile([128, C], F32, tag="osb", name=f"osb{tb}")
                    nc.vector.tensor_add(osb, ps[:, 0:512], projb_bc)
                    nc.sync.dma_start(
                        out=out_d[tb * 128:(tb + 1) * 128, :], in_=osb)

                prefetch_bias(0)
                prefetch_bias(1)
                prefetch_bias(2)
                projwT = persist.tile([128, CB, C], FP16, tag="projwT")
                nc.scalar.dma_start(
                    out=projwT,
                    in_=projwT_d.rearrange("(cb p) f -> p cb f", p=128))
                projb_bc = persist.tile([128, C], F32, tag="projb_bc")
                nc.scalar.dma_start(
                    out=projb_bc, in_=projbrow_d[:].to_broadcast((128, C)))
                qkv(0)
                qkv(1)
                norms_tensor(0)
                scaleqk(0)
                qekn_calc(0)
                vproj()
                qkv(2)
                norms_tensor(1)
                scaleqk(1)
                qekn_calc(1)
                qkv(3)
                norms_tensor(2)
                scaleqk(2)
                qekn_calc(2)
                norms_tensor(3)
                scaleqk(3)
                qekn_calc(3)
                for h in range(H):
                    if h + 3 < H:
                        prefetch_bias(h + 3)
                    group_main(h)
                    if h >= 1:
                        group_div(h - 1, 0)
                        group_div(h - 1, 1)
                group_div(H - 1, 0)
                for tb in range(NT // 2):
                    proj(tb)
                group_div(H - 1, 1)
                for tb in range(NT // 2, NT):
                    proj(tb)


    nc.compile()
    return nc


def _host_prep(inputs):
    """Host-side layout/scalar prep. Returns per-core input maps."""
    x = np.asarray(inputs["x"], dtype=np.float32)
    qkv_w = np.asarray(inputs["qkv_w"], dtype=np.float32)
    qkv_b = np.asarray(inputs["qkv_b"], dtype=np.float32)
    proj_w = np.asarray(inputs["proj_w"], dtype=np.float32)
    proj_b = np.asarray(inputs["proj_b"], dtype=np.float32)
    temp = np.asarray(inputs["temperature"], dtype=np.float32).reshape(H)
    qe = np.asarray(inputs["query_embedding"], dtype=np.float32).reshape(H, HD)
    tab = np.asarray(inputs["relative_coords_table"], dtype=np.float32)
    idx = np.asarray(inputs["relative_pos_index"])
    f1w = np.asarray(inputs["cpb_fc1_w"], dtype=np.float32)
    f1b = np.asarray(inputs["cpb_fc1_b"], dtype=np.float32)
    f2w = np.asarray(inputs["cpb_fc2_w"], dtype=np.float32)
    f2b = np.asarray(inputs["cpb_fc2_b"], dtype=np.float32)
    sls = np.asarray(inputs["seq_length_scale"], dtype=np.float32)

    scale = (np.logaddexp(0.0, temp) * sls[0]).astype(np.float32)

    hidden = np.maximum(tab @ f1w.T + f1b, 0.0)
    bias_tab = ((hidden @ f2w.T + f2b) * LOG2E).astype(np.float32)  # (T, H)
    bias = bias_tab[idx]                                            # (q, k, H)
    biasT = np.ascontiguousarray(np.transpose(bias, (2, 1, 0)))     # (H, k, q)
    biasT = biasT.astype(NB_FP8)

    wqkT = np.ascontiguousarray(qkv_w[:2 * C].T).astype(NB_FP16)   # (cin, 1024)
    wvT = np.ascontiguousarray(qkv_w[2 * C:].T).astype(NB_FP16)    # (cin, 512)
    projwT = np.ascontiguousarray(proj_w.T).astype(NB_FP16)        # (cin, 512)
    qkb = qkv_b[:2 * C].reshape(2 * C, 1).copy()
    vbrow = qkv_b[2 * C:].reshape(1, C).copy()
    projbrow = proj_b.reshape(1, C).copy()
    # qesbd[f, s]: qe*scale for feature f of the even (s=0) / odd (s=1) head
    # of f's block; knT.T @ qesbd-slice gives k_hat . qe*scale per key.
    qesbd = np.zeros((C, 2), dtype=np.float32)
    for h in range(H):
        hp, s = h // 2, h % 2
        qesbd[hp * 128 + s * 64:hp * 128 + (s + 1) * 64, s] = qe[h] * scale[h]
    qesbd = qesbd.astype(NB_FP16)
    sclq = np.ascontiguousarray(
        (scale * LOG2E).reshape(CB, 2).T).astype(np.float32)

    selb = np.zeros((2, 128), dtype=NB_FP16)
    selb[0, :64] = 1.0
    selb[1, 64:] = 1.0
    bsum = np.zeros((128, 2), dtype=NB_FP16)
    bsum[:64, 0] = 1.0
    bsum[64:, 1] = 1.0
    identf8 = np.eye(128, dtype=NB_FP8)
    ones64 = np.ones((1, 64), dtype=NB_BF16)

    shared = dict(
        wqkT=wqkT, wvT=wvT, qkb=qkb, vbrow=vbrow, qesbd=qesbd,
        sclq=sclq, projwT=projwT, projbrow=projbrow, biasT=biasT,
        selb=selb, bsum=bsum, identf8=identf8, ones64=ones64,
    )
    in_maps = []
    for b in range(B):
        m = dict(shared)
        m["xT"] = np.ascontiguousarray(x[b].T).astype(NB_FP16)
        in_maps.append(m)
    return in_maps


def get_nc(reps=1):
    key = ("nc", reps)
    if key not in _CACHE:
        _CACHE[key] = _build(reps)
    return _CACHE[key]


def kernel(**inputs) -> np.ndarray:
    nc = get_nc()
    in_maps = _host_prep(inputs)
    res = run_bass_kernel_spmd(nc, in_maps, core_ids=list(range(B)))
    out = np.stack([res.results[b]["out"] for b in range(B)], axis=0)
    return out.astype(np.float32)


# revision 42
# speedup vs baseline: 1.1775x; 1.1775x over previous
"""Self-contained Trainium2 kernel for nn_Attention_24799141167815.

Cosine-similarity attention (Swin-v2 style) with continuous position bias.
Data-parallel over batch B=8 across 8 NeuronCores (core b handles batch b).

Design (vs original baseline):
  - rel-bias is RAW (not exp'd), log2e-scaled, fp8e4, and added into the
    scores PSUM via an identity-stationary matmul at the start of each
    accumulation group; the ACT Exp(scale=ln2) converts back. This removes
    the post-exp multiply chain entirely and halves the bias DMA traffic.
  - score matmuls for the two heads of a feature block are row-packed
    (head A on partitions 0-63, head B on 64-127, issued back-to-back so
    both 64-deep matmuls run concurrently in the PE array).
  - qb-merged attention groups: the scores psum is [128, 1024] (both
    512-wide q blocks of one key tile) so each ACT Exp covers FD=1024,
    halving the per-op ACT overhead on the critical engine.
  - qsT/knT/x/weights in fp16 (not bf16) for 4x better score precision.
  - reciprocals via reciprocal_approx_fast (~1 cyc/elem, not 8/elem),
    always from SBUF (custom DVE ops misread PSUM at partition offsets).
  - emission order keeps every engine queue dense: attention for head
    pair 0 is emitted right after its dependencies, with later head
    pairs' QKV/norms interleaved between groups.
"""

import os
import numpy as np
import ml_dtypes

import concourse.bass as bass
import concourse.mybir as mybir
import concourse.tile as tile
from concourse import bacc
from concourse.bass_utils import run_bass_kernel_spmd

F32 = mybir.dt.float32
BF16 = mybir.dt.bfloat16
FP16 = mybir.dt.float16
FP8 = mybir.dt.float8e4
AF = mybir.ActivationFunctionType
ALU = mybir.AluOpType

B, N, C = 8, 1024, 512
H, HD = 8, 64
NT = N // 128     # 8 key tiles
CB = C // 128     # 4 cin blocks
QB = 2            # q blocks of 512
NB_BF16 = np.dtype(ml_dtypes.bfloat16)
NB_FP16 = np.dtype(np.float16)
NB_FP8 = np.dtype(ml_dtypes.float8_e4m3)
LOG2E = float(np.log2(np.e))
LN2 = float(np.log(2.0))

_CACHE = {}


def _build(reps=1):
    nc = bacc.Bacc("TRN2", target_bir_lowering=False)

    xT_d = nc.declare_dram_parameter("xT", [C, N], FP16, isOutput=False)
    wqkT_d = nc.declare_dram_parameter("wqkT", [C, 2 * C], FP16, isOutput=False)
    wvT_d = nc.declare_dram_parameter("wvT", [C, C], FP16, isOutput=False)
    qkb_d = nc.declare_dram_parameter("qkb", [2 * C, 1], F32, isOutput=False)
    vbrow_d = nc.declare_dram_parameter("vbrow", [1, C], F32, isOutput=False)
    qesbd_d = nc.declare_dram_parameter("qesbd", [C, 2], FP16, isOutput=False)
    sclq_d = nc.declare_dram_parameter("sclq", [2, CB], F32, isOutput=False)
    projwT_d = nc.declare_dram_parameter("projwT", [C, C], FP16, isOutput=False)
    projbrow_d = nc.declare_dram_parameter("projbrow", [1, C], F32, isOutput=False)
    biasT_d = nc.declare_dram_parameter("biasT", [H, N, N], FP8, isOutput=False)
    selb_d = nc.declare_dram_parameter("selb", [2, 128], FP16, isOutput=False)
    bsum_d = nc.declare_dram_parameter("bsum", [128, 2], FP16, isOutput=False)
    identf8_d = nc.declare_dram_parameter("identf8", [128, 128], FP8, isOutput=False)
    ones64_d = nc.declare_dram_parameter("ones64", [1, 64], BF16, isOutput=False)
    out_d = nc.declare_dram_parameter("out", [N, C], F32, isOutput=True)

    with tile.TileContext(nc) as tc:
        with (
            tc.tile_pool(name="persist", bufs=1) as persist,
            tc.tile_pool(name="sqp", bufs=2) as sqp,
            tc.tile_pool(name="ebias", bufs=3) as ebias,
            tc.tile_pool(name="expt", bufs=2) as expt_pool,
            tc.tile_pool(name="small", bufs=2) as small,
            tc.tile_pool(name="osbp", bufs=2) as osbp,
            tc.tile_pool(name="ps_big", bufs=3, space="PSUM") as ps_big,
            tc.tile_pool(name="ps_av", bufs=2, space="PSUM") as ps_av,
        ):
            # ---------------- load constants / weights ----------------
            # xT per cb chunk so the first matmul (which contracts cb=0
            # first) can start as soon as possible
            xT = persist.tile([128, CB, N], FP16, tag="xT")
            nc.sync.dma_start(
                out=xT[:, 0:2, :],
                in_=xT_d.rearrange("(cb p) n -> p cb n", p=128)[:, 0:2, :])
            nc.scalar.dma_start(
                out=xT[:, 2:4, :],
                in_=xT_d.rearrange("(cb p) n -> p cb n", p=128)[:, 2:4, :])
            wqkT = persist.tile([128, CB, 2 * C], FP16, tag="wqkT")
            wvT = persist.tile([128, CB, C], FP16, tag="wvT")
            for i, fb in enumerate([0, CB, 1, CB + 1, 2, CB + 2, 3, CB + 3]):
                nc.sync.dma_start(
                    out=wqkT[:, :, fb * 128:(fb + 1) * 128],
                    in_=wqkT_d.rearrange("(cb p) f -> p cb f", p=128)[
                        :, :, fb * 128:(fb + 1) * 128])
                if i == 1:
                    nc.sync.dma_start(
                        out=wvT,
                        in_=wvT_d.rearrange("(cb p) f -> p cb f", p=128))
            qkb = persist.tile([128, 2 * CB], F32, tag="qkb")
            nc.sync.dma_start(
                out=qkb, in_=qkb_d.rearrange("(fb p) one -> p (fb one)", p=128))
            sclq = persist.tile([2, CB], F32, tag="sclq")
            nc.sync.dma_start(out=sclq, in_=sclq_d[:])
            selb = persist.tile([2, 128], FP16, tag="selb")
            nc.sync.dma_start(out=selb, in_=selb_d[:])
            bsum = persist.tile([128, 2], FP16, tag="bsum")
            nc.sync.dma_start(out=bsum, in_=bsum_d[:])
            identf8 = persist.tile([128, 128], FP8, tag="identf8")
            nc.sync.dma_start(out=identf8, in_=identf8_d[:])
            ones64 = persist.tile([1, 64], BF16, tag="ones64")
            nc.sync.dma_start(out=ones64, in_=ones64_d[:])
            vb_bc = persist.tile([128, C], F32, tag="vb_bc")
            nc.sync.dma_start(out=vb_bc, in_=vbrow_d[:].to_broadcast((128, C)))
            qesbd = persist.tile([128, CB, 2], FP16, tag="qesbd")
            nc.sync.dma_start(
                out=qesbd, in_=qesbd_d.rearrange("(cb p) s -> p cb s", p=128))

            for rep in range(reps):
                qkT = persist.tile([128, 2 * CB, N], FP16, tag="qkT")
                qsT = persist.tile([128, CB, N], FP16, tag="qsT")
                knT = persist.tile([128, CB, N], FP16, tag="knT")
                v_sb = persist.tile([128, NT, H, HD + 1], BF16, tag="v_sb")
                qekn = persist.tile([128, CB, NT, 2], F32, tag="qekn")
                outhT = persist.tile([128, CB, N], FP16, tag="outhT")
                nrms = [None] * (2 * CB)

                def qkv(hp):
                    # q (fb=hp) and k (fb=CB+hp) projections, DVE eviction
                    for half in range(2):
                        fb = half * CB + hp
                        for qb in range(QB):
                            ps = ps_big.tile(
                                [128, 1024], F32, tag="ps_big",
                                name=f"psqkv{hp}{half}{qb}")
                            for cb in range(CB):
                                nc.tensor.matmul(
                                    ps[:, 0:512],
                                    wqkT[:, cb, fb * 128:(fb + 1) * 128],
                                    xT[:, cb, qb * 512:(qb + 1) * 512],
                                    start=(cb == 0), stop=(cb == CB - 1),
                                )
                            nc.vector.tensor_scalar(
                                out=qkT[:, fb, qb * 512:(qb + 1) * 512],
                                in0=ps[:, 0:512], scalar1=qkb[:, fb:fb + 1],
                                scalar2=None, op0=ALU.add)

                def vproj():
                    nc.vector.memset(v_sb[:, :, :, HD:HD + 1], 1.0)
                    for tb in range(NT):
                        ps = ps_big.tile([128, 1024], F32, tag="ps_big",
                                         name=f"psv{tb}")
                        for cb in range(CB):
                            nc.tensor.matmul(
                                ps[:, 0:512],
                                xT[:, cb, tb * 128:(tb + 1) * 128],
                                wvT[:, cb, :],
                                start=(cb == 0), stop=(cb == CB - 1),
                            )
                        nc.vector.tensor_add(
                            v_sb[:, tb, :, 0:HD],
                            ps[:, 0:512].rearrange("p (h d) -> p h d", h=H),
                            vb_bc.rearrange("p (h d) -> p h d", h=H),
                        )

                def norms_tensor(hp):
                    # squared sums -> sqrt -> 1/x -> clamp+scale, per q/k half
                    for half in range(2):
                        fb = half * CB + hp
                        sq = sqp.tile([128, N], FP16, tag="sq",
                                      name=f"sq{hp}{half}")
                        nc.vector.tensor_mul(sq, qkT[:, fb, :], qkT[:, fb, :])
                        nrm = sqp.tile([2, N], F32, tag="nrm", bufs=1,
                                       name=f"nrm{hp}{half}")
                        for qb in range(QB):
                            pss = ps_big.tile([128, 1024], F32, tag="ps_big",
                                              name=f"psn{hp}{half}{qb}")
                            nc.tensor.matmul(
                                pss[0:2, 0:512], bsum,
                                sq[:, qb * 512:(qb + 1) * 512],
                                start=True, stop=True)
                            nc.scalar.activation(
                                out=nrm[:, qb * 512:(qb + 1) * 512],
                                in_=pss[0:2, 0:512],
                                func=AF.Sqrt, bias=0.0, scale=1.0)
                        rr = sqp.tile([2, N], F32, tag="rr", bufs=1,
                                      name=f"rr{hp}{half}")
                        nc.vector.reciprocal_approx_fast(rr, nrm)
                        rr16 = sqp.tile([2, N], FP16, tag="rr16",
                                        name=f"rr16{hp}{half}")
                        nc.vector.tensor_scalar(
                            out=rr16, in0=rr,
                            scalar1=1e12, scalar2=(
                                sclq[:, hp:hp + 1] if half == 0 else 1.0),
                            op0=ALU.min, op1=ALU.mult)
                        nrms[2 * hp + half] = rr16

                def scaleqk(hp):
                    # broadcast norms to 128 partitions; scale features
                    for half, dst in ((0, qsT), (1, knT)):
                        fb = half * CB + hp
                        rr16 = nrms[2 * hp + half]
                        for qb in range(QB):
                            psb = ps_big.tile([128, 1024], F32, tag="ps_big",
                                              name=f"psb{hp}{half}{qb}")
                            nc.tensor.matmul(
                                psb[:, 0:512], selb,
                                rr16[:, qb * 512:(qb + 1) * 512],
                                start=True, stop=True)
                            nc.vector.tensor_mul(
                                dst[:, hp, qb * 512:(qb + 1) * 512],
                                qkT[:, fb, qb * 512:(qb + 1) * 512],
                                psb[:, 0:512])

                def qekn_calc(hp):
                    psq = ps_big.tile([128, 1024], F32, tag="ps_big",
                                      name=f"psq{hp}")
                    for kt in range(NT):
                        nc.tensor.matmul(
                            psq[:, kt * 2:kt * 2 + 2],
                            knT[:, hp, kt * 128:(kt + 1) * 128],
                            qesbd[:, hp, :],
                            start=True, stop=True)
                    nc.vector.tensor_copy(
                        qekn[:, hp, :, :].rearrange("p a s -> p (a s)"),
                        psq[:, 0:2 * NT])

                bias_tiles = {}

                def prefetch_bias(h):
                    # bias DMAs ride the scalar (ACT) hwdge queue so the big
                    # fp8 streams never block weight loads on the sync queue
                    bt = ebias.tile([128, NT, 1024], FP8, tag="bias",
                                    name=f"bias{h}")
                    nc.scalar.dma_start(
                        out=bt,
                        in_=biasT_d[h].rearrange("(kt p) q -> p kt q", p=128))
                    bias_tiles[h] = bt

                pav_tiles = {}
                eT_tiles = {}

                def group_main(h):
                    # attention for one head, both q blocks at once
                    hp, sub = h // 2, h % 2
                    po = sub * 64
                    bias_t = bias_tiles.pop(h)
                    eT = expt_pool.tile([128, NT, 1024], BF16, tag="eT",
                                        name=f"eT{h}")
                    eT_tiles[h] = eT
                    pavs = [ps_av.tile([HD + 1, 512], F32, tag="ps_av",
                                       name=f"pav{h}{i}") for i in range(2)]
                    pav_tiles[h] = pavs

                    def scores(kt):
                        ps = ps_big.tile([128, 1024], F32, tag="ps_big",
                                         name=f"pssc{h}{kt}")
                        for qb in range(QB):
                            nc.tensor.matmul(
                                ps[:, qb * 512:(qb + 1) * 512], identf8,
                                bias_t[:, kt, qb * 512:(qb + 1) * 512],
                                start=True, stop=False)
                        for qb in range(QB):
                            nc.tensor.matmul(
                                ps[:, qb * 512:(qb + 1) * 512],
                                knT[:, hp, kt * 128:(kt + 1) * 128][po:po + 64],
                                qsT[:, hp, qb * 512:(qb + 1) * 512][po:po + 64],
                                start=False, stop=True,
                            )
                        nc.scalar.activation(
                            out=eT[:, kt, :], in_=ps, func=AF.Exp,
                            bias=qekn[:, hp, kt, sub:sub + 1], scale=LN2)

                    def av(kt):
                        for qb in range(QB):
                            nc.tensor.matmul(
                                pavs[qb],
                                v_sb[:, kt, h, :],
                                eT[:, kt, qb * 512:(qb + 1) * 512],
                                start=(kt == 0), stop=(kt == NT - 1),
                            )

                    # AV lags scores by 3 kt so the previous group's division
                    # (which holds the pav slots) has time to finish
                    for kt in range(NT):
                        scores(kt)
                        if kt >= 3:
                            av(kt - 3)
                    for kt in range(NT - 3, NT):
                        av(kt)

                def group_div(h, qb):
                    # division for one (head, q-block): out = num * recip(den)
                    hp, sub = h // 2, h % 2
                    po = sub * 64
                    pav = pav_tiles[h][qb]
                    den = small.tile([1, 512], F32, tag="dens",
                                     name=f"den{h}{qb}")
                    nc.vector.tensor_copy(den, pav[HD:HD + 1, :])
                    rrec = small.tile([1, 512], F32, tag="rrec",
                                      name=f"rrec{h}{qb}")
                    nc.vector.reciprocal_approx_fast(rrec, den)
                    rrb = small.tile([HD, 512], F32, tag="rrb",
                                     name=f"rrb{h}{qb}")
                    nc.gpsimd.partition_broadcast(rrb, rrec)
                    nc.vector.scalar_tensor_tensor(
                        out=outhT[:, hp, qb * 512:(qb + 1) * 512][po:po + 64],
                        in0=pav[0:HD, :], scalar=1.0, in1=rrb,
                        op0=ALU.mult, op1=ALU.mult)

                def proj(tb):
                    ps = ps_big.tile([128, 1024], F32, tag="ps_big",
                                     name=f"pso{tb}")
                    for fb in range(CB):
                        nc.tensor.matmul(
                            ps[:, 0:512],
                            outhT[:, fb, tb * 128:(tb + 1) * 128],
                            projwT[:, fb, :],
                            start=(fb == 0), stop=(fb == CB - 1),
                        )
                    osb = osbp.tile([128, C], F32, tag="osb", name=f"osb{tb}")
                    nc.vector.tensor_add(osb, ps[:, 0:512], projb_bc)
                    nc.sync.dma_start(
                        out=out_d[tb * 128:(tb + 1) * 128, :], in_=osb)

                prefetch_bias(0)
                prefetch_bias(1)
                prefetch_bias(2)
                projwT = persist.tile([128, CB, C], FP16, tag="projwT")
                nc.scalar.dma_start(
                    out=projwT,
                    in_=projwT_d.rearrange("(cb p) f -> p cb f", p=128))
                projb_bc = persist.tile([128, C], F32, tag="projb_bc")
                nc.scalar.dma_start(
                    out=projb_bc, in_=projbrow_d[:].to_broadcast((128, C)))
                qkv(0)
                qkv(1)
                norms_tensor(0)
                scaleqk(0)
                qekn_calc(0)
                vproj()
                norms_tensor(1)
                scaleqk(1)
                qekn_calc(1)
                for h in range(H):
                    if h + 3 < H:
                        prefetch_bias(h + 3)
                    # defer later head-pairs' projections and norm chains so
                    # attention starts ~25us earlier; sqrt batches cost two
                    # extra ACT table reloads but keep the PE warm
                    if h == 1:
                        qkv(2)
                        norms_tensor(2)
                        scaleqk(2)
                        qekn_calc(2)
                    if h == 3:
                        qkv(3)
                        norms_tensor(3)
                        scaleqk(3)
                        qekn_calc(3)
                    group_main(h)
                    if h >= 1:
                        group_div(h - 1, 0)
                        group_div(h - 1, 1)
                group_div(H - 1, 0)
                for tb in range(NT // 2):
                    proj(tb)
                group_div(H - 1, 1)
                for tb in range(NT // 2, NT):
                    proj(tb)


    nc.compile()
    return nc


def _host_prep(inputs):
    """Host-side layout/scalar prep. Returns per-core input maps."""
    x = np.asarray(inputs["x"], dtype=np.float32)
    qkv_w = np.asarray(inputs["qkv_w"], dtype=np.float32)
    qkv_b = np.asarray(inputs["qkv_b"], dtype=np.float32)
    proj_w = np.asarray(inputs["proj_w"], dtype=np.float32)
    proj_b = np.asarray(inputs["proj_b"], dtype=np.float32)
    temp = np.asarray(inputs["temperature"], dtype=np.float32).reshape(H)
    qe = np.asarray(inputs["query_embedding"], dtype=np.float32).reshape(H, HD)
    tab = np.asarray(inputs["relative_coords_table"], dtype=np.float32)
    idx = np.asarray(inputs["relative_pos_index"])
    f1w = np.asarray(inputs["cpb_fc1_w"], dtype=np.float32)
    f1b = np.asarray(inputs["cpb_fc1_b"], dtype=np.float32)
    f2w = np.asarray(inputs["cpb_fc2_w"], dtype=np.float32)
    f2b = np.asarray(inputs["cpb_fc2_b"], dtype=np.float32)
    sls = np.asarray(inputs["seq_length_scale"], dtype=np.float32)

    scale = (np.logaddexp(0.0, temp) * sls[0]).astype(np.float32)

    hidden = np.maximum(tab @ f1w.T + f1b, 0.0)
    bias_tab = ((hidden @ f2w.T + f2b) * LOG2E).astype(np.float32)  # (T, H)
    bias = bias_tab[idx]                                            # (q, k, H)
    biasT = np.ascontiguousarray(np.transpose(bias, (2, 1, 0)))     # (H, k, q)
    biasT = biasT.astype(NB_FP8)

    wqkT = np.ascontiguousarray(qkv_w[:2 * C].T).astype(NB_FP16)   # (cin, 1024)
    wvT = np.ascontiguousarray(qkv_w[2 * C:].T).astype(NB_FP16)    # (cin, 512)
    projwT = np.ascontiguousarray(proj_w.T).astype(NB_FP16)        # (cin, 512)
    qkb = qkv_b[:2 * C].reshape(2 * C, 1).copy()
    vbrow = qkv_b[2 * C:].reshape(1, C).copy()
    projbrow = proj_b.reshape(1, C).copy()
    # qesbd[f, s]: qe*scale for feature f of the even (s=0) / odd (s=1) head
    # of f's block; knT.T @ qesbd-slice gives k_hat . qe*scale per key.
    qesbd = np.zeros((C, 2), dtype=np.float32)
    for h in range(H):
        hp, s = h // 2, h % 2
        qesbd[hp * 128 + s * 64:hp * 128 + (s + 1) * 64, s] = qe[h] * scale[h]
    qesbd = qesbd.astype(NB_FP16)
    sclq = np.ascontiguousarray(
        (scale * LOG2E).reshape(CB, 2).T).astype(np.float32)

    selb = np.zeros((2, 128), dtype=NB_FP16)
    selb[0, :64] = 1.0
    selb[1, 64:] = 1.0
    bsum = np.zeros((128, 2), dtype=NB_FP16)
    bsum[:64, 0] = 1.0
    bsum[64:, 1] = 1.0
    identf8 = np.eye(128, dtype=NB_FP8)
    ones64 = np.ones((1, 64), dtype=NB_BF16)

    shared = dict(
        wqkT=wqkT, wvT=wvT, qkb=qkb, vbrow=vbrow, qesbd=qesbd,
        sclq=sclq, projwT=projwT, projbrow=projbrow, biasT=biasT,
        selb=selb, bsum=bsum, identf8=identf8, ones64=ones64,
    )
    in_maps = []
    for b in range(B):
        m = dict(shared)
        m["xT"] = np.ascontiguousarray(x[b].T).astype(NB_FP16)
        in_maps.append(m)
    return in_maps


def get_nc(reps=1):
    key = ("nc", reps)
    if key not in _CACHE:
        _CACHE[key] = _build(reps)
    return _CACHE[key]


def kernel(**inputs) -> np.ndarray:
    nc = get_nc()
    in_maps = _host_prep(inputs)
    res = run_bass_kernel_spmd(nc, in_maps, core_ids=list(range(B)))
    out = np.stack([res.results[b]["out"] for b in range(B)], axis=0)
    return out.astype(np.float32)
